# revision 20
# baseline (speedup 1.0000x reference)
"""AttentionBlock (GroupNorm -> qkv -> attention -> proj -> residual) on 8 TRN2 cores.

Data-parallel over batch: B=8 samples, one per NeuronCore; no collectives.

v2: all heavy matmuls run in fp8(e4m3) with perf_mode=DoubleRow (K=256 per
instruction, ~2x bf16 PE throughput). Scale bookkeeping: weights are
pre-scaled x8 on the host so fp8 tensors sit in the format's sweet spot;
q,k,v come out as 8x their true values, scores psum = 64x, undone inside the
exp evacuation (scale=1/(64*sqrt(C)), bias=-ln8 so P is stored as P/8).
attn is stored as 32x its true value (evac multiplies by 4*recip(den/8));
the proj psum is then 256x and the epilogue applies scale=1/256.

Per-core layout (C=512 channels, T=4096 tokens):
  - x resident in SBUF f32 (4 tiles [128, T]) - used for GN stats, GN
    normalize, and the final residual; loaded from HBM exactly once.
  - DoubleRow pairs channel chunks: h, q, k stored as 2 tiles [128, 2, T]
    (fp8), vT as 16 tiles [128, 2, C], weights as [128, 2, 3C]/[128, 2, C].
  - scores computed TRANSPOSED: ST[s, t] = sum_c k[c,s] q[c,t], so softmax's
    reduction axis (s) lands on partitions. exp evacuation (ScalarE) yields
    P^T tiles (fp8) directly consumable as the PV moving operand.
  - softmax denominator: DVE accumulates the 32 P^T tiles into a bf16
    [128, TCH] buffer; one ones^T bf16 matmul reduces partitions; reciprocal
    + DRAM-round-trip broadcast to [128, TCH].
  - GroupNorm cross-partition reductions/broadcasts are done with tiny bf16
    mask matmuls on the PE (no DRAM round trips).

Weight transposes / x8 fp8 quantization / v-bias folding (softmax rows sum
to 1 => v bias contributes proj_w @ b_v + proj_b) are done on the host -
O(C^2) one-time prep.

build_graph(repeat=R) unrolls the whole per-sample computation R times
(serialized through the same SBUF buffers) - used by the timing harness to
cancel the large per-dispatch overhead of the axon/PJRT path:
device_time = (slope(R=hi) - slope(R=1)) / (hi - 1).
"""

import os
import sys

for _p in ("/opt/trn_rl_repo", "/opt/pypackages"):
    if os.path.isdir(_p) and _p not in sys.path:
        sys.path.insert(0, _p)

import numpy as np
import ml_dtypes

import json as _json

import concourse.bass as bass
import concourse.tile as tile
from concourse import mybir
from concourse.bass_utils import run_bass_kernel_spmd

# Walrus's codegen (setupSyncWait) encodes at most ONE sync wait on a DMA
# instruction and errors out ("Too many sync wait commands") instead of
# splitting. Tile's scheduler freely attaches several waits. This pass hoists
# excess waits into standalone EventSemaphore instructions on the same engine
# immediately before the offending instruction — semantically identical (the
# engine's sequencer evaluates them in stream order before issuing it).
_WAIT_LIMITS = {"DMACopy": 1}
_WAIT_LIMIT_DEFAULT = 1


def _legalize_sync_waits(raw: bytes) -> bytes:
    d = _json.loads(raw)
    n_hoisted = 0
    for fn in d.get("functions", []):
        for blk in fn.get("blocks", []):
            out = []
            for inst in blk["instructions"]:
                si = inst.get("sync_info")
                waits = (si or {}).get("on_wait") or []
                limit = _WAIT_LIMITS.get(inst.get("opcode"), _WAIT_LIMIT_DEFAULT)
                if len(waits) > limit and inst.get("engine") not in (
                        None, "Unassigned"):
                    keep = waits[-limit:]
                    hoist = waits[:-limit]
                    for j, w in enumerate(hoist):
                        out.append({
                            "debug": inst.get("debug", 0),
                            "engine": inst["engine"],
                            "ins": [], "outs": [],
                            "name": f"{inst['name']}-hw{j}",
                            "opcode": "EventSemaphore",
                            "sync_info": {"on_update": [], "on_wait": [w]},
                        })
                        n_hoisted += 1
                    si["on_wait"] = keep
                out.append(inst)
            blk["instructions"] = out
    if n_hoisted:
        d.setdefault("attributes", {})
    return _json.dumps(d).encode()


def _dedupe_ldweights(raw: bytes) -> bytes:
    """Drop PE Ldweights identical to the immediately preceding Ldweights on
    the same engine (the stationary operand is still loaded — weight
    registers persist across Matmults). Preserves any semaphore ops of the
    dropped instruction via a standalone EventSemaphore."""
    d = _json.loads(raw)
    n_dropped = 0
    for fn in d.get("functions", []):
        for blk in fn.get("blocks", []):
            last_sig = {}
            out = []
            for inst in blk["instructions"]:
                if inst.get("opcode") != "Ldweights":
                    out.append(inst)
                    continue
                eng = inst.get("engine")
                sig = _json.dumps(
                    [inst.get("ins"), inst.get("perf_mode"),
                     inst.get("tile_position"), inst.get("tile_size"),
                     inst.get("is_transpose")], sort_keys=True)
                if last_sig.get(eng) == sig:
                    si = inst.get("sync_info") or {}
                    if si.get("on_wait") or si.get("on_update"):
                        out.append({
                            "debug": inst.get("debug", 0),
                            "engine": eng, "ins": [], "outs": [],
                            "name": f"{inst['name']}-dw",
                            "opcode": "EventSemaphore",
                            "sync_info": si,
                        })
                    n_dropped += 1
                    continue
                last_sig[eng] = sig
                out.append(inst)
            blk["instructions"] = out
    return _json.dumps(d).encode()


DEDUPE_LDW = True


def _install_wait_legalizer(nc):
    orig = nc.to_json_bytes

    def patched():
        raw = orig()
        if DEDUPE_LDW:
            raw = _dedupe_ldweights(raw)
        return _legalize_sync_waits(raw)

    nc.to_json_bytes = patched


F32 = mybir.dt.float32
BF16 = mybir.dt.bfloat16
FP8 = mybir.dt.float8e4
AL = mybir.AluOpType
AF = mybir.ActivationFunctionType
DR = mybir.MatmulPerfMode.DoubleRow

C = 512
G = 32          # groupnorm groups
NCT = C // 128  # 4 channel tiles
NCP = NCT // 2  # 2 DoubleRow channel-pairs
EPS = 1e-5
TCH = 512       # t-chunk width

WS = 8.0        # fp8 weight pre-scale
ATT = 32.0      # attn storage scale
SC_EXP = 1.0 / (WS * WS * float(C) ** 0.5)   # undo 64x + C^-0.5 logit scale
PB = float(np.log(WS))                        # pt stored as P/8

ABLATE = "full"  # timing-ablation variants (ablate_time.py); "full" = real


def build_graph(T, n_cores=8, repeat=1):
    NT = T // 128    # 32 s-tiles
    NST2 = NT // 2   # 16 DoubleRow s-pairs
    NCH = T // TCH   # 8 t-chunks
    nc = bass.Bass("TRN2", target_bir_lowering=False, debug=False,
                   num_devices=n_cores)

    x_d = nc.dram_tensor("x", [C, T], F32, kind="ExternalInput").ap()
    # DoubleRow weight layout: rows (cpair*128 + p), cols (ktile*N + o)
    wqkv_d = nc.dram_tensor("wqkv", [C // 2, 2 * 3 * C], FP8,
                            kind="ExternalInput").ap()
    wproj_d = nc.dram_tensor("wproj", [C // 2, 2 * C], FP8,
                             kind="ExternalInput").ap()
    bqk_d = nc.dram_tensor("bqk", [2 * C, 1], F32, kind="ExternalInput").ap()
    bout_d = nc.dram_tensor("bout", [C, 1], F32, kind="ExternalInput").ap()
    gnw_d = nc.dram_tensor("gnw", [C, 1], F32, kind="ExternalInput").ap()
    gnb_d = nc.dram_tensor("gnb", [C, 1], F32, kind="ExternalInput").ap()
    gmask_d = nc.dram_tensor("gmask", [C, G], BF16, kind="ExternalInput").ap()
    gmaskT_d = nc.dram_tensor("gmaskT", [G, C], BF16,
                              kind="ExternalInput").ap()
    out_d = nc.dram_tensor("out", [C, T], F32, kind="ExternalOutput").ap()

    with tile.TileContext(nc) as tc:
        with (
            tc.tile_pool(name="singles", bufs=1) as sing,
            tc.tile_pool(name="persist", bufs=1) as pers,
        ):
            # ---- weights & constants (resident whole kernel) ----
            wqkv_sb = []
            for i in range(NCP):
                w = sing.tile([128, 2, 3 * C], FP8, name=f"wqkv{i}",
                              tag=f"wqkv{i}")
                nc.gpsimd.dma_start(
                    w.rearrange("p a b -> p (a b)"),
                    wqkv_d[i * 128:(i + 1) * 128, :])
                wqkv_sb.append(w)
            wproj_sb = []
            for i in range(NCP):
                w = sing.tile([128, 2, C], FP8, name=f"wproj{i}",
                              tag=f"wproj{i}")
                nc.gpsimd.dma_start(
                    w.rearrange("p a b -> p (a b)"),
                    wproj_d[i * 128:(i + 1) * 128, :])
                wproj_sb.append(w)
            bout_sb = []
            for i in range(NCT):
                b = sing.tile([128, 1], F32, name=f"bout{i}", tag=f"bout{i}")
                nc.gpsimd.dma_start(b, bout_d[i * 128:(i + 1) * 128, :])
                bout_sb.append(b)
            ones_sb = sing.tile([128, 1], FP8, name="ones", tag="ones")
            nc.vector.memset(ones_sb, 1.0)
            expb_sb = sing.tile([128, 1], F32, name="expb", tag="expb")
            nc.vector.memset(expb_sb, -PB)
            gmask_sb = sing.tile([C // 4, 4, G], BF16, name="gmask",
                                 tag="gmask")
            nc.gpsimd.dma_start(
                gmask_sb.rearrange("p a g -> p (a g)"),
                bass.AP(tensor=gmask_d.tensor, offset=gmask_d.offset,
                        ap=[[G, 128], [128 * G, 4], [1, G]]))
            gmaskT_sb = sing.tile([G, C], BF16, name="gmaskT", tag="gmaskT")
            nc.gpsimd.dma_start(gmaskT_sb, gmaskT_d)
            bq_sb, bk_sb = [], []
            for i in range(NCT):
                b = sing.tile([128, 1], F32, name=f"bq{i}", tag=f"bq{i}")
                nc.gpsimd.dma_start(b, bqk_d[i * 128:(i + 1) * 128, :])
                bq_sb.append(b)
            for i in range(NCT):
                b = sing.tile([128, 1], F32, name=f"bk{i}", tag=f"bk{i}")
                nc.gpsimd.dma_start(b, bqk_d[C + i * 128:C + (i + 1) * 128, :])
                bk_sb.append(b)
            gnw_sb, gnb_sb = [], []
            for i in range(NCT):
                b = sing.tile([128, 1], F32, name=f"gnw{i}", tag=f"gnw{i}")
                nc.gpsimd.dma_start(b, gnw_d[i * 128:(i + 1) * 128, :])
                gnw_sb.append(b)
            for i in range(NCT):
                b = sing.tile([128, 1], F32, name=f"gnb{i}", tag=f"gnb{i}")
                nc.gpsimd.dma_start(b, gnb_d[i * 128:(i + 1) * 128, :])
                gnb_sb.append(b)

            for _rep in range(repeat):
                _body(nc, tc, pers, x_d, out_d, wqkv_sb, wproj_sb, bout_sb,
                      ones_sb, expb_sb, gmask_sb, gmaskT_sb, bq_sb, bk_sb,
                      gnw_sb, gnb_sb, T, NT, NST2, NCH)
    _install_wait_legalizer(nc)
    return nc


def _body(nc, tc, pers, x_d, out_d, wqkv_sb, wproj_sb, bout_sb, ones_sb,
          expb_sb, gmask_sb, gmaskT_sb, bq_sb, bk_sb, gnw_sb, gnb_sb,
          T, NT, NST2, NCH):
    # ---- persistent activations ----
    x_sb = [pers.tile([128, T], F32, name=f"x{i}", tag=f"x{i}")
            for i in range(NCT)]
    q_sb = [pers.tile([128, 2, T], FP8, name=f"q{i}", tag=f"q{i}")
            for i in range(NCP)]
    k_sb = [pers.tile([128, 2, T], FP8, name=f"k{i}", tag=f"k{i}")
            for i in range(NCP)]
    vT_sb = [pers.tile([128, 2, C], FP8, name=f"vt{s}", tag=f"vt{s}")
             for s in range(NST2)]
    A_sb = [pers.tile([128, 1], F32, name=f"gnA{i}", tag=f"gnA{i}")
            for i in range(NCT)]
    B_sb = [pers.tile([128, 1], F32, name=f"gnB{i}", tag=f"gnB{i}")
            for i in range(NCT)]

    for ci in range(NCT):
        nc.gpsimd.dma_start(x_sb[ci], x_d[ci * 128:(ci + 1) * 128, :])

    with tc.tile_pool(name="ph", bufs=1) as ph:
        h_sb = [ph.tile([128, 2, T], FP8, name=f"h{i}", tag=f"h{i}")
                for i in range(NCP)]

        # ================= phase 1: GroupNorm =================
        with (
            tc.tile_pool(name="gns", bufs=2) as gns,
            tc.tile_pool(name="gn1", bufs=1) as gn1,
            tc.tile_pool(name="gnp", bufs=2, space="PSUM") as gnp,
        ):
            nbn = T // 512
            mvb = gn1.tile([128, 4, 2], BF16, name="mvb", tag="mvb")
            for ci in range(NCT):
                bns = gns.tile([128, nbn, 6], F32, name="bns", tag="bns")
                x3 = x_sb[ci].rearrange("p (n f) -> p n f", f=512)
                for j in range(nbn):
                    nc.vector.bn_stats(bns[:, j, :], x3[:, j, :])
                mv = gns.tile([128, 2], F32, name="mv", tag="mv")
                nc.vector.bn_aggr(mv, bns)
                # mv[:,1] <- E[x^2] = mu^2 + var
                nc.vector.scalar_tensor_tensor(
                    mv[:, 1:2], in0=mv[:, 0:1], scalar=mv[:, 0:1],
                    in1=mv[:, 1:2], op0=AL.mult, op1=AL.add)
                nc.vector.tensor_copy(mvb[:, ci, :], mv)
            # group sums across partitions: mask^T @ mv  -> [G, 2]
            gps = gnp.tile([G, 2], F32, name="gps", tag="gps")
            for ci in range(NCT):
                nc.tensor.matmul(gps, lhsT=gmask_sb[:, ci, :],
                                 rhs=mvb[:, ci, :],
                                 start=(ci == 0), stop=(ci == NCT - 1))
            gv = gn1.tile([G, 2], F32, name="gv", tag="gv")
            nc.vector.tensor_scalar_mul(gv, gps, float(G) / C)
            stdv = gn1.tile([G, 1], F32, name="stdv", tag="stdv")
            # mu^2 - E[x^2] = -var
            nc.vector.scalar_tensor_tensor(
                stdv, in0=gv[:, 0:1], scalar=gv[:, 0:1],
                in1=gv[:, 1:2], op0=AL.mult, op1=AL.subtract)
            nc.vector.tensor_scalar(stdv, stdv, -1.0, EPS,
                                    op0=AL.mult, op1=AL.add)
            nc.scalar.activation(stdv, stdv, AF.Sqrt)
            grpb = gn1.tile([G, 2], BF16, name="grpb", tag="grpb")
            rstd = gn1.tile([G, 1], F32, name="rstd", tag="rstd")
            nc.vector.reciprocal(rstd, stdv)
            nc.vector.tensor_copy(grpb[:, 0:1], rstd)
            nc.vector.tensor_copy(grpb[:, 1:2], gv[:, 0:1])
            # broadcast (rstd, mu) back per channel via maskT matmul
            for ci in range(NCT):
                pcb = gnp.tile([128, 2], F32, name="pcb", tag="pcb")
                nc.tensor.matmul(pcb,
                                 lhsT=gmaskT_sb[:, ci * 128:(ci + 1) * 128],
                                 rhs=grpb, start=True, stop=True)
                # A = gn_w * rstd ; B = gn_b - mu * A
                nc.vector.tensor_mul(A_sb[ci], gnw_sb[ci], pcb[:, 0:1])
                tmp = gns.tile([128, 1], F32, name="gn_tmp", tag="tmp")
                nc.vector.tensor_mul(tmp, pcb[:, 1:2], A_sb[ci])
                nc.vector.tensor_sub(B_sb[ci], gnb_sb[ci], tmp)
            # h = A*x + B (f32 -> fp8)
            for ci in range(NCT):
                nc.vector.tensor_scalar(
                    h_sb[ci // 2][:, ci % 2, :], x_sb[ci],
                    A_sb[ci], B_sb[ci], op0=AL.mult, op1=AL.add)

        # ================= phase 2: qkv =================
        # Loop nest ci -> cp -> ch reuses each stationary weight slice for
        # 8 consecutive matmuls (8 psum banks); the BIR ldweights-dedupe
        # pass then drops the 7 redundant weight loads.
        with tc.tile_pool(name="qkvp", bufs=1, space="PSUM") as qkvp:
            # k then q (scores need all of k first); ScalarE evacuates
            # k, DVE evacuates q — balance the two engines.
            for which, dst, bias, coff in (("k", k_sb, bk_sb, C),
                                           ("q", q_sb, bq_sb, 0)):
                for ci in range(NCT):
                    pss = [qkvp.tile([128, TCH], F32, name="qkv_ps",
                                     tag=f"ch{ch}") for ch in range(NCH)]
                    for cp in range(NCP):
                        for ch in range(NCH):
                            nc.tensor.matmul(
                                pss[ch],
                                lhsT=wqkv_sb[cp][:, :,
                                                 coff + ci * 128:
                                                 coff + (ci + 1) * 128],
                                rhs=h_sb[cp][:, :, ch * TCH:(ch + 1) * TCH],
                                start=(cp == 0), stop=(cp == NCP - 1),
                                perf_mode=DR)
                    for ch in range(NCH):
                        od = dst[ci // 2][:, ci % 2, ch * TCH:(ch + 1) * TCH]
                        if which == "k":
                            nc.scalar.activation(od, pss[ch], AF.Identity,
                                                 bias=bias[ci])
                        else:
                            nc.vector.tensor_scalar(
                                od, pss[ch], bias[ci], None, op0=AL.add)
            # vT tiles [128t, 2, C] (v bias folded into bout on host)
            for st2 in range(NST2):
                for j in range(2):
                    t0 = (2 * st2 + j) * 128
                    ps = qkvp.tile([128, C], F32, name="qkv_ps2",
                                   tag=f"ch{(st2 * 2 + j) % NCH}")
                    for cp in range(NCP):
                        nc.tensor.matmul(
                            ps,
                            lhsT=h_sb[cp][:, :, t0:t0 + 128],
                            rhs=wqkv_sb[cp][:, :, 2 * C:3 * C],
                            start=(cp == 0), stop=(cp == NCP - 1),
                            perf_mode=DR)
                    nc.vector.tensor_copy(vT_sb[st2][:, j, :], ps)

    # ================= phase 3: attention + proj =================
    with (
        tc.tile_pool(name="p3s", bufs=2) as p3s,
        tc.tile_pool(name="p3w", bufs=2) as p3w,
        tc.tile_pool(name="p3d", bufs=2, space="DRAM") as p3d,
        tc.tile_pool(name="pst", bufs=3, space="PSUM") as pst,
        tc.tile_pool(name="pden", bufs=1, space="PSUM") as pden,
        tc.tile_pool(name="ppv", bufs=2, space="PSUM") as ppv,
        tc.tile_pool(name="ppr", bufs=2, space="PSUM") as ppr,
    ):
        do_scores = ABLATE not in ("qkvonly",)
        do_den = ABLATE not in ("qkvonly", "noden")
        do_pv = ABLATE not in ("qkvonly", "upto_scores")
        do_proj = ABLATE not in ("qkvonly", "upto_scores", "upto_pv")
        DLAG = 3  # lag den matmuls behind exp so they never stall the PE

        def consume_ops(tci, pt, den_bc):
            """PV + attn evac + proj + epilogue closures for one t-chunk.

            Returned as a list of emit-callbacks so the caller can interleave
            them into the next chunk's scores loop (keeps the PE fed while
            ScalarE's exp stream catches up)."""
            t0 = tci * TCH
            ops = []
            attn = [p3w.tile([128, 2, TCH], FP8, name=f"attn{cp}",
                             tag=f"attn{cp}") for cp in range(NCP)]
            state = {}

            def pv_mm(ci, st2):
                def f():
                    if st2 == 0:
                        state[ci] = ppv.tile([128, TCH], F32, name="pv_ps",
                                             tag="pv")
                    nc.tensor.matmul(
                        state[ci],
                        lhsT=vT_sb[st2][:, :, ci * 128:(ci + 1) * 128],
                        rhs=pt[:, 2 * st2:2 * st2 + 2, :],
                        start=(st2 == 0), stop=(st2 == NST2 - 1),
                        perf_mode=DR)
                return f

            def pv_evac(ci):
                def f():
                    if do_den:
                        nc.vector.scalar_tensor_tensor(
                            attn[ci // 2][:, ci % 2, :], in0=state[ci],
                            scalar=ATT / WS, in1=den_bc,
                            op0=AL.mult, op1=AL.mult)
                    else:
                        nc.vector.tensor_scalar(
                            attn[ci // 2][:, ci % 2, :], state[ci],
                            ATT / WS, None, op0=AL.mult)
                return f

            for ci in range(NCT if do_pv else 0):
                ops.extend(pv_mm(ci, st2) for st2 in range(NST2))
                ops.append(pv_evac(ci))

            def proj_one(oi):
                def f():
                    pr = ppr.tile([128, TCH], F32, name="pr_ps", tag="pr")
                    for cp in range(NCP if do_proj else 0):
                        nc.tensor.matmul(
                            pr,
                            lhsT=wproj_sb[cp][:, :, oi * 128:(oi + 1) * 128],
                            rhs=attn[cp],
                            start=(cp == 0), stop=(cp == NCP - 1),
                            perf_mode=DR)
                    osb = p3w.tile([128, TCH], F32, name="osb", tag="osb",
                                   bufs=3)
                    if do_proj:
                        nc.scalar.activation(osb, pr, AF.Identity,
                                             bias=bout_sb[oi],
                                             scale=1.0 / (WS * ATT))
                        nc.vector.tensor_add(osb, osb,
                                             x_sb[oi][:, t0:t0 + TCH])
                    else:
                        nc.vector.tensor_copy(osb, x_sb[oi][:, t0:t0 + TCH])
                    nc.gpsimd.dma_start(
                        out_d[oi * 128:(oi + 1) * 128, t0:t0 + TCH], osb)
                return f

            ops.extend(proj_one(oi) for oi in range(NCT))
            return ops

        # software-pipelined: scores/exp/den of chunk tci interleave with
        # PV/proj of chunk tci-1 at instruction granularity (the PE always
        # has filler work while ScalarE's exp stream catches up, and the den
        # DMA round trip never stalls anything)
        fill = []
        for tci in range(NCH if do_scores else 0):
            t0 = tci * TCH
            # scores^T + exp -> PT tiles [s,t] in SBUF (fp8, = P/8)
            pt = p3s.tile([128, NT, TCH], FP8, name="pt", tag="pt")
            dps = pden.tile([1, TCH], F32, name="den_ps", tag="den")

            def den_mm(st, dps=dps, pt=pt):
                nc.tensor.matmul(dps, lhsT=ones_sb, rhs=pt[:, st, :],
                                 start=(st == 0), stop=(st == NT - 1))

            nfill = (len(fill) + NT - 1) // NT
            for st in range(NT):
                sp = pst.tile([128, TCH], F32, name="st_ps", tag="st")
                for cp in range(NCP):
                    nc.tensor.matmul(
                        sp,
                        lhsT=k_sb[cp][:, :, st * 128:(st + 1) * 128],
                        rhs=q_sb[cp][:, :, t0:t0 + TCH],
                        start=(cp == 0), stop=(cp == NCP - 1),
                        perf_mode=DR)
                nc.scalar.activation(pt[:, st, :], sp, AF.Exp,
                                     bias=expb_sb, scale=SC_EXP)
                if do_den and st >= DLAG:
                    den_mm(st - DLAG)
                for op in fill[st * nfill:(st + 1) * nfill]:
                    op()
            fill = fill[NT * nfill:]
            # den tail + reciprocal + DRAM-round-trip partition broadcast
            den_bc = None
            if do_den:
                for st in range(NT - DLAG, NT):
                    den_mm(st)
                den_bc = p3w.tile([128, TCH], F32, name="den_bc",
                                  tag="den_bc")
                den = p3w.tile([1, TCH], F32, name="den_sb", tag="den_sb")
                nc.vector.reciprocal(den, dps)
                dscr = p3d.tile([1, TCH], F32, name="dscr", tag="dscr")
                nc.gpsimd.dma_start(dscr, den)
                dsrc = bass.AP(tensor=dscr.tensor, offset=dscr.offset,
                               ap=[[0, 128], [1, TCH]])
                nc.gpsimd.dma_start(den_bc, dsrc)
            fill = fill + consume_ops(tci, pt, den_bc)
        for op in fill:
            op()


def _fp8(a):
    return np.clip(a, -240.0, 240.0).astype(ml_dtypes.float8_e4m3)


def _dr_layout(wt, n):
    """[C, n] transposed weight -> DoubleRow DRAM layout [C//2, 2*n].

    Row (cpair*128 + p), col (ktile*n + o) holds wt[cpair*256 + ktile*128
    + p, o].
    """
    return np.ascontiguousarray(
        wt.reshape(NCP, 2, 128, n).transpose(0, 2, 1, 3).reshape(C // 2,
                                                                 2 * n))


def host_prep(gn_w, gn_b, qkv_w, qkv_b, proj_w, proj_b):
    """One-time O(C^2) weight prep in numpy -> per-core replicated inputs."""
    w8 = _fp8(qkv_w * WS).astype(np.float32)   # quantize once, exact layout
    wqkvt = _dr_layout(np.ascontiguousarray(w8.T), 3 * C)
    wp8 = _fp8(proj_w * WS).astype(np.float32)
    wprojt = _dr_layout(np.ascontiguousarray(wp8.T), C)
    bqk = (qkv_b[:2 * C] * WS).astype(np.float32).reshape(2 * C, 1)
    bout = (proj_w @ qkv_b[2 * C:] + proj_b).astype(np.float32).reshape(C, 1)
    gsize = C // G
    ch_group = np.arange(C) // gsize
    gmask = (ch_group[:, None] == np.arange(G)[None, :])
    return {
        "wqkv": _fp8(wqkvt), "wproj": _fp8(wprojt), "bqk": bqk, "bout": bout,
        "gnw": gn_w.astype(np.float32).reshape(C, 1),
        "gnb": gn_b.astype(np.float32).reshape(C, 1),
        "gmask": gmask.astype(ml_dtypes.bfloat16),
        "gmaskT": np.ascontiguousarray(gmask.T).astype(ml_dtypes.bfloat16),
    }


_graph_cache = {}


def run(x, gn_w, gn_b, qkv_w, qkv_b, proj_w, proj_b, trace=False):
    x = np.asarray(x, np.float32)
    B, Cv, H, W = x.shape
    T = H * W
    shared = host_prep(np.asarray(gn_w), np.asarray(gn_b),
                       np.asarray(qkv_w), np.asarray(qkv_b),
                       np.asarray(proj_w), np.asarray(proj_b))
    key = (T, B)
    if key not in _graph_cache:
        _graph_cache[key] = build_graph(T, n_cores=B)
    nc = _graph_cache[key]
    in_maps = []
    for i in range(B):
        m = dict(shared)
        m["x"] = np.ascontiguousarray(x[i].reshape(Cv, T))
        in_maps.append(m)
    try:
        res = run_bass_kernel_spmd(nc, in_maps, core_ids=list(range(B)),
                                   trace=trace)
    except ModuleNotFoundError:
        # axon NTFF profiling hook unavailable in this container
        res = run_bass_kernel_spmd(nc, in_maps, core_ids=list(range(B)),
                                   trace=False)
    out = np.stack([res.results[i]["out"] for i in range(B)])
    return out.reshape(B, Cv, H, W).astype(np.float32), res


def kernel(**inputs):
    out, _ = run(**inputs)
    return out
